# revision 7
# baseline (speedup 1.0000x reference)
"""Bass/Tile TRN2 kernel for nn_BertSelfAttention2 (B=2, S=2048, D=1024, H=16).

Sharding: 8 cores = 2 (batch) x 4 (head groups of 4 heads). Each core
computes Q/K projections for its 4 heads (as 2 packed pairs), the modified
attention (kt = softplus(k), v = q + k, mask on the query axis), and writes
its [S, 256] slice of the output.

Layout trick: everything is computed in "T" orientation (scoresT[k, q]) so
no large on-device transposes are needed. The query-axis mask is applied by
zeroing masked query columns of Q; softmax of an all-zero score column then
reproduces the reference's uniform-probability behaviour for masked queries
exactly. The softmax denominator comes from a ones-column appended to V.
"""
import sys

if "/opt/trn_rl_repo" not in sys.path:
    sys.path.insert(0, "/opt/trn_rl_repo")

import numpy as np

B, S, D = 2, 2048, 1024
H = 16
HD = 64
NCORES = 8
HPC = H // (NCORES // B)     # heads per core = 4
NG = HPC // 2                # head-pair groups per core = 2
SC = 4                       # 512-wide query chunks
KC = S // 128                # 16 key chunks
SUPER = 2                    # key chunks per exp supertile

_CACHE = {}


def _build():
    import concourse.tile as tile
    from concourse import bacc, mybir
    from concourse.masks import make_identity

    F32 = mybir.dt.float32
    F32R = mybir.dt.float32r
    AF = mybir.ActivationFunctionType

    nc = bacc.Bacc(None, target_bir_lowering=False, debug=False)

    xt = nc.declare_dram_parameter("xt", [D, S], F32R, isOutput=False)
    wq = nc.declare_dram_parameter("wq", [D, 2 * 128], F32R, isOutput=False)
    wk = nc.declare_dram_parameter("wk", [D, 2 * 128], F32R, isOutput=False)
    bq = nc.declare_dram_parameter("bq", [2 * 128], F32, isOutput=False)
    bk = nc.declare_dram_parameter("bk", [2 * 128], F32, isOutput=False)
    maskb = nc.declare_dram_parameter("maskb", [128, S], F32, isOutput=False)
    ones16 = nc.declare_dram_parameter("ones16", [128, KC], F32R, isOutput=False)
    out = nc.declare_dram_parameter("out", [S, 2 * 128], F32, isOutput=True)

    with tile.TileContext(nc) as tc:
        with tc.tile_pool(name="consts", bufs=1) as consts, \
             tc.tile_pool(name="big", bufs=1) as big, \
             tc.tile_pool(name="tmp", bufs=4) as tmp, \
             tc.tile_pool(name="expp", bufs=2) as expp, \
             tc.tile_pool(name="ep", bufs=2) as ep, \
             tc.tile_pool(name="ps_s", bufs=1, space="PSUM") as ps_s, \
             tc.tile_pool(name="ps_c", bufs=1, space="PSUM") as ps_c, \
             tc.tile_pool(name="ps_m", bufs=2, space="PSUM") as ps_m:

            ident = consts.tile([128, 128], F32)
            make_identity(nc, ident)

            mask_t = consts.tile([128, S], F32)
            nc.sync.dma_start(out=mask_t, in_=maskb[:, :])

            bq_t, bk_t = [], []
            for g in range(NG):
                bqt = consts.tile([128, 1], F32, tag=f"bq{g}")
                nc.sync.dma_start(
                    out=bqt, in_=bq[g * 128:(g + 1) * 128].rearrange("(p o) -> p o", o=1))
                bq_t.append(bqt)
                bkt = consts.tile([128, 1], F32, tag=f"bk{g}")
                nc.sync.dma_start(
                    out=bkt, in_=bk[g * 128:(g + 1) * 128].rearrange("(p o) -> p o", o=1))
                bk_t.append(bkt)

            # weight tiles [d_chunk 128, 128 (2 heads x 64)]
            wq_t = [[consts.tile([128, 128], F32R, tag=f"wq{g}_{dchunk}", name=f"wq{g}_{dchunk}")
                     for dchunk in range(8)] for g in range(NG)]
            wk_t = [[consts.tile([128, 128], F32R, tag=f"wk{g}_{dchunk}", name=f"wk{g}_{dchunk}")
                     for dchunk in range(8)] for g in range(NG)]
            for g in range(NG):
                for dc in range(8):
                    nc.sync.dma_start(
                        out=wq_t[g][dc],
                        in_=wq[dc * 128:(dc + 1) * 128, g * 128:(g + 1) * 128])
                    nc.sync.dma_start(
                        out=wk_t[g][dc],
                        in_=wk[dc * 128:(dc + 1) * 128, g * 128:(g + 1) * 128])

            # X^T tiles: [128 (d), 2048 (s)] x 8
            xt_t = [big.tile([128, S], F32R, tag=f"xt{dchunk}", name=f"xt{dchunk}") for dchunk in range(8)]
            for dc in range(8):
                nc.sync.dma_start(out=xt_t[dc], in_=xt[dc * 128:(dc + 1) * 128, :])

            # persistent activations
            qt = [big.tile([128, S], F32R, tag=f"qt{g}", name=f"qt{g}") for g in range(NG)]
            kt = [big.tile([128, S], F32R, tag=f"kt{g}", name=f"kt{g}") for g in range(NG)]
            vt = [big.tile([128, S], F32, tag=f"vt{g}", name=f"vt{g}") for g in range(NG)]
            # V' natural layout per head: [128 (keys), 16 chunks, 64+ones]
            vp = [big.tile([128, KC, 65], F32R, tag=f"vp{h}", name=f"vp{h}") for h in range(HPC)]
            for h in range(HPC):
                nc.sync.dma_start(
                    out=vp[h][:, :, 64:65],
                    in_=ones16[:, :].rearrange("p (c o) -> p c o", o=1))

            # ---- phase 1: projections ----
            for g in range(NG):
                for sc in range(SC):
                    ssl = slice(sc * 512, (sc + 1) * 512)
                    pq = ps_s.tile([128, 1024], F32, tag="sA")
                    for dc in range(8):
                        nc.tensor.matmul(pq[:, 0:512], wq_t[g][dc],
                                         xt_t[dc][:, ssl],
                                         start=(dc == 0), stop=(dc == 7))
                    pk = ps_s.tile([128, 1024], F32, tag="sB")
                    for dc in range(8):
                        nc.tensor.matmul(pk[:, 0:512], wk_t[g][dc],
                                         xt_t[dc][:, ssl],
                                         start=(dc == 0), stop=(dc == 7))
                    tq = tmp.tile([128, 512], F32, tag="tq")
                    nc.vector.tensor_scalar_add(tq, pq[:, 0:512], bq_t[g])
                    tk = tmp.tile([128, 512], F32, tag="tk")
                    nc.vector.tensor_scalar_add(tk, pk[:, 0:512], bk_t[g])
                    # v = q + k (raw)
                    nc.vector.tensor_add(vt[g][:, ssl], tq, tk)
                    # masked q for scores
                    nc.vector.tensor_mul(qt[g][:, ssl], tq, mask_t[:, ssl])
                    # kt = softplus(k) = ln(exp(k) + 1)
                    te = tmp.tile([128, 512], F32, tag="te")
                    nc.scalar.activation(out=te, in_=tk, func=AF.Exp)
                    nc.scalar.activation(out=kt[g][:, ssl], in_=te, func=AF.Ln,
                                         bias=1.0)

            # ---- phase 1b: V' = transpose(vt) per head ----
            for g in range(NG):
                for hh in range(2):
                    h = g * 2 + hh
                    hsl = slice(hh * 64, (hh + 1) * 64)
                    isl = slice(hh * 64, (hh + 1) * 64)
                    for j in range(KC):
                        pv = ps_m.tile([128, 65], F32, tag="ep")
                        nc.tensor.transpose(pv[:, 0:64],
                                            vt[g][hsl, j * 128:(j + 1) * 128],
                                            ident[isl, isl])
                        nc.vector.tensor_copy(vp[h][:, j, 0:64], pv[:, 0:64])

            # ---- phase 2: attention ----
            for g in range(NG):
                qtA = qt[g][0:64, :]
                qtB = qt[g][64:128, :]
                ktA = kt[g][0:64, :]
                ktB = kt[g][64:128, :]
                vpA = vp[g * 2]
                vpB = vp[g * 2 + 1]
                for qc in range(SC):
                    qsl = slice(qc * 512, (qc + 1) * 512)
                    cA = ps_c.tile([65, 512], F32, tag="cA")
                    cB = ps_c.tile([65, 512], F32, tag="cB")
                    for st in range(KC // SUPER):
                        sA = ps_s.tile([128, SUPER * 512], F32, tag="sA")
                        sB = ps_s.tile([128, SUPER * 512], F32, tag="sB")
                        for kk in range(SUPER):
                            kc = st * SUPER + kk
                            ksl = slice(kc * 128, (kc + 1) * 128)
                            osl = slice(kk * 512, (kk + 1) * 512)
                            nc.tensor.matmul(sA[:, osl], ktA[:, ksl], qtA[:, qsl],
                                             start=True, stop=True)
                            nc.tensor.matmul(sB[:, osl], ktB[:, ksl], qtB[:, qsl],
                                             start=True, stop=True)
                        eA = expp.tile([128, SUPER * 512], F32R, tag="eA")
                        nc.scalar.activation(out=eA, in_=sA, func=AF.Exp, scale=0.125)
                        eB = expp.tile([128, SUPER * 512], F32R, tag="eB")
                        nc.scalar.activation(out=eB, in_=sB, func=AF.Exp, scale=0.125)
                        for kk in range(SUPER):
                            kc = st * SUPER + kk
                            osl = slice(kk * 512, (kk + 1) * 512)
                            nc.tensor.matmul(cA, vpA[:, kc, :], eA[:, osl],
                                             start=(kc == 0), stop=(kc == KC - 1))
                            nc.tensor.matmul(cB, vpB[:, kc, :], eB[:, osl],
                                             start=(kc == 0), stop=(kc == KC - 1))
                    # epilogue: transpose ctxT back, normalize, store
                    csA = ep.tile([65, 512], F32, tag="csA")
                    nc.vector.tensor_copy(csA, cA)
                    csB = ep.tile([65, 512], F32, tag="csB")
                    nc.vector.tensor_copy(csB, cB)
                    for j in range(4):
                        jsl = slice(j * 128, (j + 1) * 128)
                        ptA = ps_m.tile([128, 65], F32, tag="ep")
                        nc.tensor.transpose(ptA[:, :], csA[:, jsl], ident[0:65, 0:65])
                        ptB = ps_m.tile([128, 65], F32, tag="ep")
                        nc.tensor.transpose(ptB[:, :], csB[:, jsl], ident[0:65, 0:65])
                        rA = ep.tile([128, 1], F32, tag="rA")
                        nc.vector.reciprocal(rA, ptA[:, 64:65])
                        rB = ep.tile([128, 1], F32, tag="rB")
                        nc.vector.reciprocal(rB, ptB[:, 64:65])
                        cf = ep.tile([128, 128], F32, tag="cf")
                        nc.vector.tensor_scalar_mul(cf[:, 0:64], ptA[:, 0:64], rA)
                        nc.vector.tensor_scalar_mul(cf[:, 64:128], ptB[:, 0:64], rB)
                        nc.sync.dma_start(
                            out=out[qc * 512 + j * 128: qc * 512 + (j + 1) * 128,
                                    g * 128:(g + 1) * 128],
                            in_=cf)

    nc.finalize()
    return nc


def _get_nc():
    if "nc" not in _CACHE:
        _CACHE["nc"] = _build()
    return _CACHE["nc"]


def _shard_inputs(hidden_states, attention_mask, Wq, bq, Wk, bk):
    hs = np.asarray(hidden_states, dtype=np.float32)
    am = np.asarray(attention_mask)
    Wq = np.asarray(Wq, dtype=np.float32)
    Wk = np.asarray(Wk, dtype=np.float32)
    bq = np.asarray(bq, dtype=np.float32)
    bk = np.asarray(bk, dtype=np.float32)

    xts = [np.ascontiguousarray(hs[b].T) for b in range(B)]
    maskbs = [np.ascontiguousarray(
        np.broadcast_to(am[b].astype(np.float32)[None, :], (128, S)))
        for b in range(B)]

    in_maps = []
    for c in range(NCORES):
        b = c // (NCORES // B)
        hg = c % (NCORES // B)
        cols = slice(hg * 2 * 128, (hg + 1) * 2 * 128)
        in_maps.append({
            "xt": xts[b],
            "wq": np.ascontiguousarray(Wq[:, cols]),
            "wk": np.ascontiguousarray(Wk[:, cols]),
            "bq": np.ascontiguousarray(bq[cols]),
            "bk": np.ascontiguousarray(bk[cols]),
            "maskb": maskbs[b],
            "ones16": np.ones((128, KC), dtype=np.float32),
        })
    return in_maps


def _gather(results):
    full = np.empty((B, S, D), dtype=np.float32)
    for c in range(NCORES):
        b = c // (NCORES // B)
        hg = c % (NCORES // B)
        cols = slice(hg * 2 * 128, (hg + 1) * 2 * 128)
        full[b, :, cols] = results[c]["out"]
    return full


def run_sharded(in_maps, **kw):
    from concourse.bass_utils import run_bass_kernel_spmd
    nc = _get_nc()
    return run_bass_kernel_spmd(nc, in_maps, list(range(NCORES)), **kw)


def kernel(hidden_states, attention_mask, Wq, bq, Wk, bk):
    in_maps = _shard_inputs(hidden_states, attention_mask, Wq, bq, Wk, bk)
    res = run_sharded(in_maps)
    return _gather(res.results)


# revision 10
# speedup vs baseline: 1.0219x; 1.0219x over previous
"""Bass/Tile TRN2 kernel for nn_BertSelfAttention2 (B=2, S=2048, D=1024, H=16).

Sharding: 8 cores = 2 (batch) x 4 (head groups of 4 heads). Each core
computes Q/K projections for its 4 heads (as 2 packed pairs), the modified
attention (kt = softplus(k), v = q + k, mask on the query axis), and writes
its [S, 256] slice of the output.

Layout trick: everything is computed in "T" orientation (scoresT[k, q]) so
no large on-device transposes are needed. The query-axis mask is applied by
zeroing masked query columns of Q; softmax of an all-zero score column then
reproduces the reference's uniform-probability behaviour for masked queries
exactly. The softmax denominator comes from a ones-column appended to V.
"""
import sys

if "/opt/trn_rl_repo" not in sys.path:
    sys.path.insert(0, "/opt/trn_rl_repo")

import numpy as np

B, S, D = 2, 2048, 1024
H = 16
HD = 64
NCORES = 8
HPC = H // (NCORES // B)     # heads per core = 4
NG = HPC // 2                # head-pair groups per core = 2
SC = 4                       # 512-wide query chunks
KC = S // 128                # 16 key chunks
SUPER = 2                    # key chunks per exp supertile

_CACHE = {}


def _build():
    import concourse.tile as tile
    from concourse import bacc, mybir
    from concourse.masks import make_identity

    F32 = mybir.dt.float32
    F32R = mybir.dt.float32r
    AF = mybir.ActivationFunctionType

    nc = bacc.Bacc(None, target_bir_lowering=False, debug=False)

    xt = nc.declare_dram_parameter("xt", [D, S], F32R, isOutput=False)
    wq = nc.declare_dram_parameter("wq", [D, 2 * 128], F32R, isOutput=False)
    wk = nc.declare_dram_parameter("wk", [D, 2 * 128], F32R, isOutput=False)
    bq = nc.declare_dram_parameter("bq", [2 * 128], F32, isOutput=False)
    bk = nc.declare_dram_parameter("bk", [2 * 128], F32, isOutput=False)
    maskb = nc.declare_dram_parameter("maskb", [128, S], F32, isOutput=False)
    ones16 = nc.declare_dram_parameter("ones16", [128, KC], F32R, isOutput=False)
    out = nc.declare_dram_parameter("out", [S, 2 * 128], F32, isOutput=True)

    with tile.TileContext(nc) as tc:
        with tc.tile_pool(name="consts", bufs=1) as consts, \
             tc.tile_pool(name="big", bufs=1) as big, \
             tc.tile_pool(name="tmp", bufs=4) as tmp, \
             tc.tile_pool(name="expp", bufs=2) as expp, \
             tc.tile_pool(name="ep", bufs=2) as ep, \
             tc.tile_pool(name="ps_s", bufs=1, space="PSUM") as ps_s, \
             tc.tile_pool(name="ps_c", bufs=1, space="PSUM") as ps_c, \
             tc.tile_pool(name="ps_m", bufs=2, space="PSUM") as ps_m:

            ident = consts.tile([128, 128], F32)
            make_identity(nc, ident)

            # X^T tiles first — the projection's critical path.  Split each
            # row-block load in half so the first chunks land early.
            xt_t = [big.tile([128, S], F32R, tag=f"xt{dchunk}", name=f"xt{dchunk}")
                    for dchunk in range(8)]
            for dc in range(8):
                nc.sync.dma_start(out=xt_t[dc][:, 0:S // 2],
                                  in_=xt[dc * 128:(dc + 1) * 128, 0:S // 2])
                nc.sync.dma_start(out=xt_t[dc][:, S // 2:S],
                                  in_=xt[dc * 128:(dc + 1) * 128, S // 2:S])

            mask_t = consts.tile([128, S], F32)
            nc.sync.dma_start(out=mask_t, in_=maskb[:, :])

            bq_t, bk_t = [], []
            for g in range(NG):
                bqt = consts.tile([128, 1], F32, tag=f"bq{g}")
                nc.sync.dma_start(
                    out=bqt, in_=bq[g * 128:(g + 1) * 128].rearrange("(p o) -> p o", o=1))
                bq_t.append(bqt)
                bkt = consts.tile([128, 1], F32, tag=f"bk{g}")
                nc.sync.dma_start(
                    out=bkt, in_=bk[g * 128:(g + 1) * 128].rearrange("(p o) -> p o", o=1))
                bk_t.append(bkt)

            # weight tiles [d_chunk 128, 128 (2 heads x 64)]
            wq_t = [[consts.tile([128, 128], F32R, tag=f"wq{g}_{dchunk}", name=f"wq{g}_{dchunk}")
                     for dchunk in range(8)] for g in range(NG)]
            wk_t = [[consts.tile([128, 128], F32R, tag=f"wk{g}_{dchunk}", name=f"wk{g}_{dchunk}")
                     for dchunk in range(8)] for g in range(NG)]
            for g in range(NG):
                for dc in range(8):
                    nc.sync.dma_start(
                        out=wq_t[g][dc],
                        in_=wq[dc * 128:(dc + 1) * 128, g * 128:(g + 1) * 128])
                    nc.sync.dma_start(
                        out=wk_t[g][dc],
                        in_=wk[dc * 128:(dc + 1) * 128, g * 128:(g + 1) * 128])

            # persistent activations
            qt = [big.tile([128, S], F32R, tag=f"qt{g}", name=f"qt{g}") for g in range(NG)]
            kt = [big.tile([128, S], F32R, tag=f"kt{g}", name=f"kt{g}") for g in range(NG)]
            vt = [big.tile([128, S], F32, tag=f"vt{g}", name=f"vt{g}") for g in range(NG)]
            # V' natural layout per head: [128 (keys), 16 chunks, 64+ones]
            vp = [big.tile([128, KC, 65], F32R, tag=f"vp{h}", name=f"vp{h}") for h in range(HPC)]
            for h in range(HPC):
                nc.sync.dma_start(
                    out=vp[h][:, :, 64:65],
                    in_=ones16[:, :].rearrange("p (c o) -> p c o", o=1))

            # ---- phase 1: projections ----
            for g in range(NG):
                for sc in range(SC):
                    ssl = slice(sc * 512, (sc + 1) * 512)
                    pq = ps_s.tile([128, 1024], F32, tag="sA")
                    for dc in range(8):
                        nc.tensor.matmul(pq[:, 0:512], wq_t[g][dc],
                                         xt_t[dc][:, ssl],
                                         start=(dc == 0), stop=(dc == 7))
                    pk = ps_s.tile([128, 1024], F32, tag="sB")
                    for dc in range(8):
                        nc.tensor.matmul(pk[:, 0:512], wk_t[g][dc],
                                         xt_t[dc][:, ssl],
                                         start=(dc == 0), stop=(dc == 7))
                    tq = tmp.tile([128, 512], F32, tag="tq")
                    nc.vector.tensor_scalar_add(tq, pq[:, 0:512], bq_t[g])
                    tk = tmp.tile([128, 512], F32, tag="tk")
                    nc.vector.tensor_scalar_add(tk, pk[:, 0:512], bk_t[g])
                    # v = q + k (raw)
                    nc.vector.tensor_add(vt[g][:, ssl], tq, tk)
                    # masked q for scores
                    nc.vector.tensor_mul(qt[g][:, ssl], tq, mask_t[:, ssl])
                    # kt = softplus(k) = ln(exp(k) + 1)
                    te = tmp.tile([128, 512], F32, tag="te")
                    nc.scalar.activation(out=te, in_=tk, func=AF.Exp)
                    nc.scalar.activation(out=kt[g][:, ssl], in_=te, func=AF.Ln,
                                         bias=1.0)
                    # V' = transpose(vt) for this s-chunk, both heads —
                    # interleaved here so the PE never starves at the
                    # proj->attention boundary (a >3.4us gap re-throttles HAM).
                    for hh in range(2):
                        h = g * 2 + hh
                        hsl = slice(hh * 64, (hh + 1) * 64)
                        for j in range(sc * 4, (sc + 1) * 4):
                            pv = ps_m.tile([128, 65], F32, tag="ep")
                            nc.tensor.transpose(pv[:, 0:64],
                                                vt[g][hsl, j * 128:(j + 1) * 128],
                                                ident[hsl, hsl])
                            nc.vector.tensor_copy(vp[h][:, j, 0:64], pv[:, 0:64])

            # ---- phase 2: attention ----
            for g in range(NG):
                qtA = qt[g][0:64, :]
                qtB = qt[g][64:128, :]
                ktA = kt[g][0:64, :]
                ktB = kt[g][64:128, :]
                vpA = vp[g * 2]
                vpB = vp[g * 2 + 1]
                for qc in range(SC):
                    qsl = slice(qc * 512, (qc + 1) * 512)
                    cA = ps_c.tile([65, 512], F32, tag="cA")
                    cB = ps_c.tile([65, 512], F32, tag="cB")
                    for st in range(KC // SUPER):
                        sA = ps_s.tile([128, SUPER * 512], F32, tag="sA")
                        sB = ps_s.tile([128, SUPER * 512], F32, tag="sB")
                        for kk in range(SUPER):
                            kc = st * SUPER + kk
                            ksl = slice(kc * 128, (kc + 1) * 128)
                            osl = slice(kk * 512, (kk + 1) * 512)
                            nc.tensor.matmul(sA[:, osl], ktA[:, ksl], qtA[:, qsl],
                                             start=True, stop=True)
                            nc.tensor.matmul(sB[:, osl], ktB[:, ksl], qtB[:, qsl],
                                             start=True, stop=True)
                        eA = expp.tile([128, SUPER * 512], F32R, tag="eA")
                        nc.scalar.activation(out=eA, in_=sA, func=AF.Exp, scale=0.125)
                        eB = expp.tile([128, SUPER * 512], F32R, tag="eB")
                        nc.scalar.activation(out=eB, in_=sB, func=AF.Exp, scale=0.125)
                        for kk in range(SUPER):
                            kc = st * SUPER + kk
                            osl = slice(kk * 512, (kk + 1) * 512)
                            nc.tensor.matmul(cA, vpA[:, kc, :], eA[:, osl],
                                             start=(kc == 0), stop=(kc == KC - 1))
                            nc.tensor.matmul(cB, vpB[:, kc, :], eB[:, osl],
                                             start=(kc == 0), stop=(kc == KC - 1))
                    # epilogue: transpose ctxT back, normalize, store
                    csA = ep.tile([65, 512], F32, tag="csA")
                    nc.vector.tensor_copy(csA, cA)
                    csB = ep.tile([65, 512], F32, tag="csB")
                    nc.vector.tensor_copy(csB, cB)
                    for j in range(4):
                        jsl = slice(j * 128, (j + 1) * 128)
                        ptA = ps_m.tile([128, 65], F32, tag="ep")
                        nc.tensor.transpose(ptA[:, :], csA[:, jsl], ident[0:65, 0:65])
                        ptB = ps_m.tile([128, 65], F32, tag="ep")
                        nc.tensor.transpose(ptB[:, :], csB[:, jsl], ident[0:65, 0:65])
                        rA = ep.tile([128, 1], F32, tag="rA")
                        nc.vector.reciprocal(rA, ptA[:, 64:65])
                        rB = ep.tile([128, 1], F32, tag="rB")
                        nc.vector.reciprocal(rB, ptB[:, 64:65])
                        cf = ep.tile([128, 128], F32, tag="cf")
                        nc.vector.tensor_scalar_mul(cf[:, 0:64], ptA[:, 0:64], rA)
                        nc.vector.tensor_scalar_mul(cf[:, 64:128], ptB[:, 0:64], rB)
                        nc.sync.dma_start(
                            out=out[qc * 512 + j * 128: qc * 512 + (j + 1) * 128,
                                    g * 128:(g + 1) * 128],
                            in_=cf)

    nc.finalize()
    return nc


def _get_nc():
    if "nc" not in _CACHE:
        _CACHE["nc"] = _build()
    return _CACHE["nc"]


def _shard_inputs(hidden_states, attention_mask, Wq, bq, Wk, bk):
    hs = np.asarray(hidden_states, dtype=np.float32)
    am = np.asarray(attention_mask)
    Wq = np.asarray(Wq, dtype=np.float32)
    Wk = np.asarray(Wk, dtype=np.float32)
    bq = np.asarray(bq, dtype=np.float32)
    bk = np.asarray(bk, dtype=np.float32)

    xts = [np.ascontiguousarray(hs[b].T) for b in range(B)]
    maskbs = [np.ascontiguousarray(
        np.broadcast_to(am[b].astype(np.float32)[None, :], (128, S)))
        for b in range(B)]

    in_maps = []
    for c in range(NCORES):
        b = c // (NCORES // B)
        hg = c % (NCORES // B)
        cols = slice(hg * 2 * 128, (hg + 1) * 2 * 128)
        in_maps.append({
            "xt": xts[b],
            "wq": np.ascontiguousarray(Wq[:, cols]),
            "wk": np.ascontiguousarray(Wk[:, cols]),
            "bq": np.ascontiguousarray(bq[cols]),
            "bk": np.ascontiguousarray(bk[cols]),
            "maskb": maskbs[b],
            "ones16": np.ones((128, KC), dtype=np.float32),
        })
    return in_maps


def _gather(results):
    full = np.empty((B, S, D), dtype=np.float32)
    for c in range(NCORES):
        b = c // (NCORES // B)
        hg = c % (NCORES // B)
        cols = slice(hg * 2 * 128, (hg + 1) * 2 * 128)
        full[b, :, cols] = results[c]["out"]
    return full


def run_sharded(in_maps, **kw):
    from concourse.bass_utils import run_bass_kernel_spmd
    nc = _get_nc()
    return run_bass_kernel_spmd(nc, in_maps, list(range(NCORES)), **kw)


def kernel(hidden_states, attention_mask, Wq, bq, Wk, bk):
    in_maps = _shard_inputs(hidden_states, attention_mask, Wq, bq, Wk, bk)
    res = run_sharded(in_maps)
    return _gather(res.results)
